# revision 5
# baseline (speedup 1.0000x reference)
"""Trainium2 Bass kernel for the ARqGPSFull autoregressive wavefunction.

Math: out[b] = sum_{s,m} ctx[b,s,m] * I_{x[b,s]}[s,m]; in logs the masked
product is affine in x, so with centered x' = x-0.5 and folded observed-state
selection (see _host_pack):
  T[b,s,m] = exp(Sre) * (cos(Sim) + i sin(Sim)),  S = x'^T D + C  (one matmul)
Per core: 8 of the 64 m-values -> 512 (s,m) columns.  D is shipped as a
bf16 hi+lo pair (two accumulating matmuls, 66+64 contraction rows) so the
matmul is fp32-exact; constants ride rows 64/65 of the hi block split hi/lo.

Engines: exp comes from the ACT Exp table (1e-5 rel); sin from the ACT Sin
table (2e-7) after one mid-kernel table-set switch.  cos(Sim) = 1-2*sin^2
(Sim/2) needs no range reduction (|Sim|<2pi); sin(Sim) = -sin(ths) with
ths = Sim + pi - 2pi*[Sim>0] in [-pi,pi].  Products and the (s,m) sums fuse
into two scalar_tensor_tensor ops with accum_out; Sum(exp) falls out of the
Exp op's accumulator, so the real part is aE - Sum(2 sq^2 pe) on the host.
Outputs (3 per-sample scalars) are stream-transposed on DVE so the output
DMA is 12 contiguous 128B rows instead of 128 tiny descriptors.

Post-compile IR surgery (same trick as before): input DMAs (Pool-queue,
25ns issue each) and the first act-table load are hoisted into the preamble
so they overlap the fixed ~6us engine startup; the second act-table load
(Sin set) stays mid-body.
"""

import sys

for _p in ("/opt/trn_rl_repo", "/root/.axon_site/_ro/trn_rl_repo"):
    if _p not in sys.path:
        sys.path.append(_p)

import math
import numpy as np
import ml_dtypes

N_CORES = 8
B = 128        # batch
L = 64         # n_sites
M = 64         # GPS support dim
NM = M // N_CORES   # m-values per core
NBLK = L * NM  # 512 (s,m) columns per core
PI = math.pi

_BF16 = ml_dtypes.bfloat16

_built = None

# rhs column layout: [REH(512) | XT(128) | REL(512) | IMH(512) | IML(512)]
C_REH = 0
C_XT = C_REH + NBLK
C_REL = C_XT + B
C_IMH = C_REL + NBLK
C_IML = C_IMH + NBLK
C_TOT = C_IML + NBLK


def _build():
    import concourse.bacc as bacc
    import concourse.mybir as mybir
    from concourse import tile

    f32 = mybir.dt.float32
    bf16 = mybir.dt.bfloat16
    AF = mybir.ActivationFunctionType
    ALU = mybir.AluOpType

    nc = bacc.Bacc()
    rhs_d = nc.dram_tensor("rhs", [66, C_TOT], bf16, kind="ExternalInput")
    out_d = nc.dram_tensor("out", [3, 4, 32], f32, kind="ExternalOutput")

    with tile.TileContext(nc) as tc:
        with (
            tc.tile_pool(name="pc", bufs=1) as pc,
            tc.tile_pool(name="psum", bufs=1, space="PSUM") as psum,
        ):
            rhs = pc.tile([66, C_TOT], bf16, tag="rhs")
            # four input DMAs; ordered so the exp path (re blocks) lands
            # first.  Issued on the Pool queue (25ns each vs 565 on SP) and
            # hoisted pre-barrier post-compile.
            nc.gpsimd.dma_start(rhs[:, C_REH:C_REL], rhs_d[:, C_REH:C_REL])
            nc.gpsimd.dma_start(rhs[:, C_REL:C_IMH], rhs_d[:, C_REL:C_IMH])
            nc.gpsimd.dma_start(rhs[:, C_IMH:C_IML], rhs_d[:, C_IMH:C_IML])
            nc.gpsimd.dma_start(rhs[:, C_IML:C_TOT], rhs_d[:, C_IML:C_TOT])
            xt66 = rhs[:, C_XT:C_XT + B]
            xt64 = rhs[0:64, C_XT:C_XT + B]

            o = pc.tile([B, 32], f32, tag="o")
            nc.gpsimd.memset(o[:], 0.0)

            # S = [Sre(512) | Sim(512)] in PSUM; hi (66 rows, with consts)
            # + lo (64 rows) accumulate per 256-col group.
            S = psum.tile([B, 2 * NBLK], f32, tag="S")
            HW = NBLK // 2
            for g in range(2):   # re chunks first: exp starts earliest
                cs, ce = g * HW, (g + 1) * HW
                nc.tensor.matmul(S[:, cs:ce], xt66,
                                 rhs[:, C_REH + cs:C_REH + ce],
                                 start=True, stop=False)
                nc.tensor.matmul(S[:, cs:ce], xt64,
                                 rhs[0:64, C_REL + cs:C_REL + ce],
                                 start=False, stop=True)
            for g in range(2):
                cs, ce = g * HW, (g + 1) * HW
                nc.tensor.matmul(S[:, NBLK + cs:NBLK + ce], xt66,
                                 rhs[:, C_IMH + cs:C_IMH + ce],
                                 start=True, stop=False)
                nc.tensor.matmul(S[:, NBLK + cs:NBLK + ce], xt64,
                                 rhs[0:64, C_IML + cs:C_IML + ce],
                                 start=False, stop=True)
            Sre = S[:, 0:NBLK]
            Sim = S[:, NBLK:2 * NBLK]

            # pe = exp(Sre), aE = sum(pe) free from the ACT accumulator
            pe = pc.tile([B, NBLK], f32, tag="pe")
            nc.scalar.activation(pe[:], Sre, AF.Exp, accum_out=o[:, 0:1])

            # range reduction for sin: ths = Sim + pi - 2pi*[Sim>0]
            w2s = pc.tile([B, NBLK], f32, tag="w2s")
            ths = pc.tile([B, NBLK], f32, tag="ths")
            for g in range(2):
                cs, ce = g * HW, (g + 1) * HW
                nc.vector.tensor_scalar(w2s[:, cs:ce], Sim[:, cs:ce],
                                        0.0, -2 * PI,
                                        op0=ALU.is_gt, op1=ALU.mult)
                nc.vector.scalar_tensor_tensor(
                    ths[:, cs:ce], Sim[:, cs:ce], PI, w2s[:, cs:ce],
                    op0=ALU.add, op1=ALU.add)

            # table switch (inserted by bacc before the first Sin)
            sq = pc.tile([B, NBLK], f32, tag="sq")
            nc.scalar.activation(sq[:], Sim, AF.Sin, scale=0.5)
            sn = pc.tile([B, NBLK], f32, tag="sn")
            nc.scalar.activation(sn[:], ths[:], AF.Sin)

            sqsq = pc.tile([B, NBLK], f32, tag="sqsq")
            nc.vector.tensor_mul(sqsq[:], sq[:], sq[:])

            # o1 = sum(pe * sn) = -Tim ; o2 = sum(2 sq^2 pe) = aE - Tre
            scrI = pc.tile([B, NBLK], f32, tag="scrI")
            nc.vector.scalar_tensor_tensor(
                scrI[:], pe[:], 1.0, sn[:], op0=ALU.mult, op1=ALU.mult,
                accum_out=o[:, 1:2])
            scrR = pc.tile([B, NBLK], f32, tag="scrR")
            nc.vector.scalar_tensor_tensor(
                scrR[:], sqsq[:], 2.0, pe[:], op0=ALU.mult, op1=ALU.mult,
                accum_out=o[:, 2:3])

            # block-transpose so the output DMA rows are contiguous:
            # tr[32k+c, p] = o[32k+p, c]
            tr = pc.tile([B, 32], f32, tag="tr")
            nc.vector.transpose(tr[:], o[:])
            # one DMA per quantity row c, partitions {c, c+32, c+64,
            # c+96}, each on its own engine queue: same-queue DMAs of
            # equal shape get mis-merged (dropped transfers)
            for c, eng in enumerate((nc.gpsimd, nc.sync, nc.scalar)):
                eng.dma_start(out_d[c], tr[c:c + 97:32, :])

    nc.compile()

    # Hoist the (wait-free) input DMAs and the FIRST act-table load (Exp
    # set) into the preamble block so they overlap the fixed engine
    # startup.  The mid-kernel Sin-set load stays in the body.
    import os
    mybir_ET = mybir.EngineType
    b0, b1 = nc.main_func.blocks[0], nc.main_func.blocks[1]
    if os.environ.get("NO_HOIST") == "1":
        return nc
    hoist = []
    first_load_seen = False
    for ins in list(b1.instructions):
        nm = type(ins).__name__
        if nm == "InstDMACopy" and ins.engine == mybir_ET.Pool:
            si = ins.sync_info
            if si is not None and si.on_wait:
                continue  # output DMAs wait on results
            hoist.append(ins)
            b1.instructions.remove(ins)
        elif nm == "InstLoadActFuncSet" and not first_load_seen:
            first_load_seen = True
            si = ins.sync_info
            assert si is None or (not si.on_wait and not si.on_update)
            hoist.append(ins)
            b1.instructions.remove(ins)
    for ins in reversed(hoist):
        first = next((i for i, x in enumerate(b0.instructions)
                      if x.engine == ins.engine), len(b0.instructions))
        b0.instructions.insert(first, ins)
    return nc


def _host_pack(inputs, params_context, inputs_param):
    x = np.asarray(inputs).astype(np.float64)          # (B, L) in {0,1}
    P = np.asarray(params_context)                     # (s, d, m, j) complex
    I = np.asarray(inputs_param)                       # (s, d, m) complex

    mask = (np.arange(L)[None, :] < np.maximum(np.arange(L), 1)[:, None])
    Lp = np.log(P)
    D = (Lp[:, 1] - Lp[:, 0]) * mask[:, None, :]       # (s, m, j)
    C = (Lp[:, 0] * mask[:, None, :]).sum(-1)          # (s, m)
    I0 = I[:, 0]
    I1 = I[:, 1]
    A0 = np.log(np.abs(I0))
    dA = np.log(np.abs(I1)) - A0
    wrap = lambda t: np.angle(np.exp(1j * t))
    ph0 = np.angle(I0)
    dPh = wrap(np.angle(I1) - ph0)
    eye = np.eye(L)[:, None, :]                        # (s, 1, j)
    Dre = D.real + eye * dA[:, :, None]                # (s, m, j)
    Dim = D.imag + eye * dPh[:, :, None]
    CA = C.real + A0 + 0.5 * Dre.sum(-1)               # x-centering shift
    PH = wrap(C.imag + ph0 + 0.5 * Dim.sum(-1))

    xt = np.concatenate([(x - 0.5).T, np.ones((2, B))], 0)  # (66, B)
    rhs_list = []
    for k in range(N_CORES):
        msl = slice(k * NM, (k + 1) * NM)
        full = np.zeros((66, C_TOT), np.float64)
        full[:, C_XT:C_XT + B] = xt
        for Dp, const, chi, clo in ((Dre, CA, C_REH, C_REL),
                                    (Dim, PH, C_IMH, C_IML)):
            Dc = Dp[:, msl, :].transpose(2, 0, 1).reshape(L, NBLK)  # (j, sm)
            Dhi = Dc.astype(_BF16).astype(np.float64)
            full[0:64, chi:chi + NBLK] = Dhi
            full[0:64, clo:clo + NBLK] = Dc - Dhi
            cc = const[:, msl].reshape(NBLK)
            hi = cc.astype(_BF16).astype(np.float64)
            full[64, chi:chi + NBLK] = hi
            full[65, chi:chi + NBLK] = cc - hi
        rhs_list.append(full.astype(_BF16))
    return rhs_list


def kernel(inputs, params_context, inputs_param):
    global _built
    from concourse.bass_utils import run_bass_kernel_spmd

    if _built is None:
        _built = _build()
    nc = _built

    rhs_list = _host_pack(inputs, params_context, inputs_param)
    in_maps = [{"rhs": rhs_list[k]} for k in range(N_CORES)]
    res = run_bass_kernel_spmd(nc, in_maps, list(range(N_CORES)))

    re = np.zeros(B, np.float64)
    im = np.zeros(B, np.float64)
    for k in range(N_CORES):
        q = np.asarray(res.results[k]["out"], np.float64)  # (3, 4, 32)
        aE = q[0].reshape(B)
        sIm = q[1].reshape(B)
        sRA = q[2].reshape(B)
        re += aE - sRA
        im += -sIm
    return (re + 1j * np.angle(np.exp(1j * im))).astype(np.complex128)


# revision 7
# speedup vs baseline: 1.0722x; 1.0722x over previous
"""Trainium2 Bass kernel for the ARqGPSFull autoregressive wavefunction.

Math: out[b] = sum_{s,m} ctx[b,s,m] * I_{x[b,s]}[s,m]; in logs the masked
product is affine in x, so with centered x' = x-0.5 and folded observed-state
selection (see _host_pack):
  T[b,s,m] = exp(Sre) * (cos(Sim) + i sin(Sim)),  S = x'^T D + C  (one matmul)
Per core: 8 of the 64 m-values -> 512 (s,m) columns.  D is shipped as a
bf16 hi+lo pair (two accumulating matmuls, 66+64 contraction rows) so the
matmul is fp32-exact; constants ride rows 64/65 of the hi block split hi/lo.

Engines: exp comes from the ACT Exp table (1e-5 rel); sin from the ACT Sin
table (2e-7) after one mid-kernel table-set switch.  cos(Sim) = 1-2*sin^2
(Sim/2) needs no range reduction (|Sim|<2pi); sin(Sim) = -sin(ths) with
ths = Sim + pi - 2pi*[Sim>0] in [-pi,pi].  Products and the (s,m) sums fuse
into two scalar_tensor_tensor ops with accum_out; Sum(exp) falls out of the
Exp op's accumulator, so the real part is aE - Sum(2 sq^2 pe) on the host.
Outputs (3 per-sample scalars) are stream-transposed on DVE so the output
DMA is 12 contiguous 128B rows instead of 128 tiny descriptors.

Post-compile IR surgery (same trick as before): input DMAs (Pool-queue,
25ns issue each) and the first act-table load are hoisted into the preamble
so they overlap the fixed ~6us engine startup; the second act-table load
(Sin set) stays mid-body.
"""

import sys

for _p in ("/opt/trn_rl_repo", "/root/.axon_site/_ro/trn_rl_repo"):
    if _p not in sys.path:
        sys.path.append(_p)

import math
import numpy as np
import ml_dtypes

N_CORES = 8
B = 128        # batch
L = 64         # n_sites
M = 64         # GPS support dim
NM = M // N_CORES   # m-values per core
NBLK = L * NM  # 512 (s,m) columns per core
PI = math.pi

_BF16 = ml_dtypes.bfloat16

_built = None

# rhs column layout: [REH(512) | XT(128) | REL(512) | IMH(512) | IML(512)]
C_REH = 0
C_XT = C_REH + NBLK
C_REL = C_XT + B
C_IMH = C_REL + NBLK
C_IML = C_IMH + NBLK
C_TOT = C_IML + NBLK


def _build():
    import concourse.bacc as bacc
    import concourse.mybir as mybir
    from concourse import tile

    f32 = mybir.dt.float32
    bf16 = mybir.dt.bfloat16
    AF = mybir.ActivationFunctionType
    ALU = mybir.AluOpType

    nc = bacc.Bacc()
    rhs_d = nc.dram_tensor("rhs", [66, C_TOT], bf16, kind="ExternalInput")
    out_d = nc.dram_tensor("out", [2, 4, 32], f32, kind="ExternalOutput")

    with tile.TileContext(nc) as tc:
        with (
            tc.tile_pool(name="pc", bufs=1) as pc,
            tc.tile_pool(name="psum", bufs=1, space="PSUM") as psum,
        ):
            rhs = pc.tile([66, C_TOT], bf16, tag="rhs")
            # four input DMAs; ordered so the exp path (re blocks) lands
            # first.  Issued on the Pool queue (25ns each vs 565 on SP) and
            # hoisted pre-barrier post-compile.
            nc.gpsimd.dma_start(rhs[:, C_REH:C_IMH], rhs_d[:, C_REH:C_IMH])
            nc.gpsimd.dma_start(rhs[:, C_IMH:C_TOT], rhs_d[:, C_IMH:C_TOT])
            xt66 = rhs[:, C_XT:C_XT + B]
            xt64 = rhs[0:64, C_XT:C_XT + B]

            o = pc.tile([B, 32], f32, tag="o")
            nc.gpsimd.memset(o[:], 0.0)

            # S = [Sre(512) | Sim(512)] in PSUM; hi (66 rows, with consts)
            # + lo (64 rows) accumulate per 256-col group.  Emission order
            # mirrors execution order: the tile tick assignment follows it,
            # and out-of-order emission inflates cross-engine waits.
            S = psum.tile([B, 2 * NBLK], f32, tag="S")
            HW = NBLK // 2
            Sre = S[:, 0:NBLK]
            Sim = S[:, NBLK:2 * NBLK]
            pe = pc.tile([B, NBLK], f32, tag="pe")
            w2s = pc.tile([B, NBLK], f32, tag="w2s")
            ths = pc.tile([B, NBLK], f32, tag="ths")
            for g in range(2):   # re chunks first: exp starts earliest
                cs, ce = g * HW, (g + 1) * HW
                nc.tensor.matmul(S[:, cs:ce], xt66,
                                 rhs[:, C_REH + cs:C_REH + ce],
                                 start=True, stop=False)
                nc.tensor.matmul(S[:, cs:ce], xt64,
                                 rhs[0:64, C_REL + cs:C_REL + ce],
                                 start=False, stop=True)
            # pe = exp(Sre), aE = sum(pe) free from the ACT accumulator
            nc.scalar.activation(pe[:], Sre, AF.Exp, accum_out=o[:, 0:1])
            for g in range(2):
                cs, ce = g * HW, (g + 1) * HW
                nc.tensor.matmul(S[:, NBLK + cs:NBLK + ce], xt66,
                                 rhs[:, C_IMH + cs:C_IMH + ce],
                                 start=True, stop=False)
                nc.tensor.matmul(S[:, NBLK + cs:NBLK + ce], xt64,
                                 rhs[0:64, C_IML + cs:C_IML + ce],
                                 start=False, stop=True)
                # range reduction: ths = Sim + pi - 2pi*[Sim>0]
                nc.vector.tensor_scalar(w2s[:, cs:ce], Sim[:, cs:ce],
                                        0.0, -2 * PI,
                                        op0=ALU.is_gt, op1=ALU.mult)
                nc.vector.scalar_tensor_tensor(
                    ths[:, cs:ce], Sim[:, cs:ce], PI, w2s[:, cs:ce],
                    op0=ALU.add, op1=ALU.add)

            # table switch (inserted by bacc before the first Sin)
            sq = pc.tile([B, NBLK], f32, tag="sq")
            nc.scalar.activation(sq[:], Sim, AF.Sin, scale=0.5)
            sn = pc.tile([B, NBLK], f32, tag="sn")
            nc.scalar.activation(sn[:], ths[:], AF.Sin)

            sqsq = pc.tile([B, NBLK], f32, tag="sqsq")
            nc.vector.tensor_mul(sqsq[:], sq[:], sq[:])

            # o1 = sum(pe * sn) = -Tim ; o2 = sum(2 sq^2 pe) = aE - Tre
            scrI = pc.tile([B, NBLK], f32, tag="scrI")
            nc.vector.scalar_tensor_tensor(
                scrI[:], pe[:], 1.0, sn[:], op0=ALU.mult, op1=ALU.mult,
                accum_out=o[:, 1:2])
            scrR = pc.tile([B, NBLK], f32, tag="scrR")
            nc.vector.scalar_tensor_tensor(
                scrR[:], sqsq[:], 2.0, pe[:], op0=ALU.mult, op1=ALU.mult,
                accum_out=o[:, 2:3])

            # oRe = aE - sum(2 sq^2 pe) on-device -> only 2 output rows
            nc.vector.tensor_sub(o[:, 3:4], o[:, 0:1], o[:, 2:3])
            # block-transpose so the output DMA rows are contiguous:
            # tr[32k+c, p] = o[32k+p, c]
            tr = pc.tile([B, 32], f32, tag="tr")
            nc.vector.transpose(tr[:], o[:])
            # one DMA per quantity row, on different queues: same-queue
            # DMAs of equal shape get mis-merged (dropped transfers)
            nc.gpsimd.dma_start(out_d[0], tr[3:100:32, :])
            nc.sync.dma_start(out_d[1], tr[1:98:32, :])

    nc.compile()

    # Hoist the (wait-free) input DMAs and the FIRST act-table load (Exp
    # set) into the preamble block so they overlap the fixed engine
    # startup.  The mid-kernel Sin-set load stays in the body.
    import os
    mybir_ET = mybir.EngineType
    b0, b1 = nc.main_func.blocks[0], nc.main_func.blocks[1]
    if os.environ.get("NO_HOIST") == "1":
        return nc
    hoist = []
    first_load_seen = False
    for ins in list(b1.instructions):
        nm = type(ins).__name__
        if nm == "InstDMACopy" and ins.engine == mybir_ET.Pool:
            si = ins.sync_info
            if si is not None and si.on_wait:
                continue  # output DMAs wait on results
            hoist.append(ins)
            b1.instructions.remove(ins)
        elif nm == "InstLoadActFuncSet" and not first_load_seen:
            first_load_seen = True
            si = ins.sync_info
            assert si is None or (not si.on_wait and not si.on_update)
            hoist.append(ins)
            b1.instructions.remove(ins)
    for ins in reversed(hoist):
        first = next((i for i, x in enumerate(b0.instructions)
                      if x.engine == ins.engine), len(b0.instructions))
        b0.instructions.insert(first, ins)
    if os.environ.get("KEEP_DRAIN") != "1":
        for ins in list(b0.instructions):
            if (type(ins).__name__ == "InstDrain"
                    and ins.engine == mybir_ET.Pool):
                b0.instructions.remove(ins)
    return nc


def _host_pack(inputs, params_context, inputs_param):
    x = np.asarray(inputs).astype(np.float64)          # (B, L) in {0,1}
    P = np.asarray(params_context)                     # (s, d, m, j) complex
    I = np.asarray(inputs_param)                       # (s, d, m) complex

    mask = (np.arange(L)[None, :] < np.maximum(np.arange(L), 1)[:, None])
    Lp = np.log(P)
    D = (Lp[:, 1] - Lp[:, 0]) * mask[:, None, :]       # (s, m, j)
    C = (Lp[:, 0] * mask[:, None, :]).sum(-1)          # (s, m)
    I0 = I[:, 0]
    I1 = I[:, 1]
    A0 = np.log(np.abs(I0))
    dA = np.log(np.abs(I1)) - A0
    wrap = lambda t: np.angle(np.exp(1j * t))
    ph0 = np.angle(I0)
    dPh = wrap(np.angle(I1) - ph0)
    eye = np.eye(L)[:, None, :]                        # (s, 1, j)
    Dre = D.real + eye * dA[:, :, None]                # (s, m, j)
    Dim = D.imag + eye * dPh[:, :, None]
    CA = C.real + A0 + 0.5 * Dre.sum(-1)               # x-centering shift
    PH = wrap(C.imag + ph0 + 0.5 * Dim.sum(-1))

    xt = np.concatenate([(x - 0.5).T, np.ones((2, B))], 0)  # (66, B)
    rhs_list = []
    for k in range(N_CORES):
        msl = slice(k * NM, (k + 1) * NM)
        full = np.zeros((66, C_TOT), np.float64)
        full[:, C_XT:C_XT + B] = xt
        for Dp, const, chi, clo in ((Dre, CA, C_REH, C_REL),
                                    (Dim, PH, C_IMH, C_IML)):
            Dc = Dp[:, msl, :].transpose(2, 0, 1).reshape(L, NBLK)  # (j, sm)
            Dhi = Dc.astype(_BF16).astype(np.float64)
            full[0:64, chi:chi + NBLK] = Dhi
            full[0:64, clo:clo + NBLK] = Dc - Dhi
            cc = const[:, msl].reshape(NBLK)
            hi = cc.astype(_BF16).astype(np.float64)
            full[64, chi:chi + NBLK] = hi
            full[65, chi:chi + NBLK] = cc - hi
        rhs_list.append(full.astype(_BF16))
    return rhs_list


def kernel(inputs, params_context, inputs_param):
    global _built
    from concourse.bass_utils import run_bass_kernel_spmd

    if _built is None:
        _built = _build()
    nc = _built

    rhs_list = _host_pack(inputs, params_context, inputs_param)
    in_maps = [{"rhs": rhs_list[k]} for k in range(N_CORES)]
    res = run_bass_kernel_spmd(nc, in_maps, list(range(N_CORES)))

    re = np.zeros(B, np.float64)
    im = np.zeros(B, np.float64)
    for k in range(N_CORES):
        q = np.asarray(res.results[k]["out"], np.float64)  # (2, 4, 32)
        re += q[0].reshape(B)
        im += -q[1].reshape(B)
    return (re + 1j * np.angle(np.exp(1j * im))).astype(np.complex128)


# revision 8
# speedup vs baseline: 1.1287x; 1.0527x over previous
"""Trainium2 Bass kernel for the ARqGPSFull autoregressive wavefunction.

Math: out[b] = sum_{s,m} ctx[b,s,m] * I_{x[b,s]}[s,m]; in logs the masked
product is affine in x, so with centered x' = x-0.5 and folded observed-state
selection (see _host_pack):
  T[b,s,m] = exp(Sre) * (cos(Sim) + i sin(Sim)),  S = x'^T D + C  (one matmul)
Per core: 8 of the 64 m-values -> 512 (s,m) columns.  D is shipped as a
bf16 hi+lo pair (two accumulating matmuls, 66+64 contraction rows) so the
matmul is fp32-exact; constants ride rows 64/65 of the hi block split hi/lo.

Engines: exp comes from the ACT Exp table (1e-5 rel); sin from the ACT Sin
table (2e-7) after one mid-kernel table-set switch.  cos(Sim) = 1-2*sin^2
(Sim/2) needs no range reduction (|Sim|<2pi); sin(Sim) = -sin(ths) with
ths = Sim + pi - 2pi*[Sim>0] in [-pi,pi].  Products and the (s,m) sums fuse
into two scalar_tensor_tensor ops with accum_out; Sum(exp) falls out of the
Exp op's accumulator, so the real part is aE - Sum(2 sq^2 pe) on the host.
Outputs (3 per-sample scalars) are stream-transposed on DVE so the output
DMA is 12 contiguous 128B rows instead of 128 tiny descriptors.

Post-compile IR surgery (same trick as before): input DMAs (Pool-queue,
25ns issue each) and the first act-table load are hoisted into the preamble
so they overlap the fixed ~6us engine startup; the second act-table load
(Sin set) stays mid-body.
"""

import sys

for _p in ("/opt/trn_rl_repo", "/root/.axon_site/_ro/trn_rl_repo"):
    if _p not in sys.path:
        sys.path.append(_p)

import math
import numpy as np
import ml_dtypes

N_CORES = 8
B = 128        # batch
L = 64         # n_sites
M = 64         # GPS support dim
NM = M // N_CORES   # m-values per core
NBLK = L * NM  # 512 (s,m) columns per core
PI = math.pi

_BF16 = ml_dtypes.bfloat16

_built = None

# rhs column layout: [REH(512) | XT(128) | REL(512) | IMH(512) | IML(512)]
C_REH = 0
C_XT = C_REH + NBLK
C_REL = C_XT + B
C_IMH = C_REL + NBLK
C_IML = C_IMH + NBLK
C_TOT = C_IML + NBLK


def _build():
    import concourse.bacc as bacc
    import concourse.mybir as mybir
    from concourse import tile

    f32 = mybir.dt.float32
    bf16 = mybir.dt.bfloat16
    AF = mybir.ActivationFunctionType
    ALU = mybir.AluOpType

    nc = bacc.Bacc()
    rhs_d = nc.dram_tensor("rhs", [66, C_TOT], bf16, kind="ExternalInput")
    out_d = nc.dram_tensor("out", [2, 4, 32], f32, kind="ExternalOutput")

    with tile.TileContext(nc) as tc:
        with (
            tc.tile_pool(name="pc", bufs=1) as pc,
            tc.tile_pool(name="psum", bufs=1, space="PSUM") as psum,
        ):
            rhs = pc.tile([66, C_TOT], bf16, tag="rhs")
            # four input DMAs; ordered so the exp path (re blocks) lands
            # first.  Issued on the Pool queue (25ns each vs 565 on SP) and
            # hoisted pre-barrier post-compile.
            nc.gpsimd.dma_start(rhs[:, C_REH:C_REL], rhs_d[:, C_REH:C_REL])
            nc.gpsimd.dma_start(rhs[:, C_REL:C_IMH], rhs_d[:, C_REL:C_IMH])
            nc.sync.dma_start(rhs[:, C_IMH:C_TOT], rhs_d[:, C_IMH:C_TOT])
            xt66 = rhs[:, C_XT:C_XT + B]
            xt64 = rhs[0:64, C_XT:C_XT + B]

            o = pc.tile([B, 32], f32, tag="o")
            nc.gpsimd.memset(o[:], 0.0)

            # S = [Sre(512) | Sim(512)] in PSUM; hi (66 rows, with consts)
            # + lo (64 rows) accumulate per 256-col group.  Emission order
            # mirrors execution order: the tile tick assignment follows it,
            # and out-of-order emission inflates cross-engine waits.
            S = psum.tile([B, 2 * NBLK], f32, tag="S")
            HW = NBLK // 2
            Sre = S[:, 0:NBLK]
            Sim = S[:, NBLK:2 * NBLK]
            pe = pc.tile([B, NBLK], f32, tag="pe")
            w2s = pc.tile([B, NBLK], f32, tag="w2s")
            ths = pc.tile([B, NBLK], f32, tag="ths")
            for g in range(2):   # re chunks first: exp starts earliest
                cs, ce = g * HW, (g + 1) * HW
                nc.tensor.matmul(S[:, cs:ce], xt66,
                                 rhs[:, C_REH + cs:C_REH + ce],
                                 start=True, stop=False)
                nc.tensor.matmul(S[:, cs:ce], xt64,
                                 rhs[0:64, C_REL + cs:C_REL + ce],
                                 start=False, stop=True)
            # pe = exp(Sre), aE = sum(pe) free from the ACT accumulator
            nc.scalar.activation(pe[:], Sre, AF.Exp, accum_out=o[:, 0:1])
            for g in range(2):
                cs, ce = g * HW, (g + 1) * HW
                nc.tensor.matmul(S[:, NBLK + cs:NBLK + ce], xt66,
                                 rhs[:, C_IMH + cs:C_IMH + ce],
                                 start=True, stop=False)
                nc.tensor.matmul(S[:, NBLK + cs:NBLK + ce], xt64,
                                 rhs[0:64, C_IML + cs:C_IML + ce],
                                 start=False, stop=True)
                # range reduction: ths = Sim + pi - 2pi*[Sim>0]
                nc.vector.tensor_scalar(w2s[:, cs:ce], Sim[:, cs:ce],
                                        0.0, -2 * PI,
                                        op0=ALU.is_gt, op1=ALU.mult)
                nc.vector.scalar_tensor_tensor(
                    ths[:, cs:ce], Sim[:, cs:ce], PI, w2s[:, cs:ce],
                    op0=ALU.add, op1=ALU.add)

            # table switch (inserted by bacc before the first Sin)
            sq = pc.tile([B, NBLK], f32, tag="sq")
            nc.scalar.activation(sq[:], Sim, AF.Sin, scale=0.5)
            sn = pc.tile([B, NBLK], f32, tag="sn")
            nc.scalar.activation(sn[:], ths[:], AF.Sin)

            sqsq = pc.tile([B, NBLK], f32, tag="sqsq")
            nc.vector.tensor_mul(sqsq[:], sq[:], sq[:])

            # o1 = sum(pe * sn) = -Tim ; o2 = sum(2 sq^2 pe) = aE - Tre
            scrI = pc.tile([B, NBLK], f32, tag="scrI")
            nc.vector.scalar_tensor_tensor(
                scrI[:], pe[:], 1.0, sn[:], op0=ALU.mult, op1=ALU.mult,
                accum_out=o[:, 1:2])
            scrR = pc.tile([B, NBLK], f32, tag="scrR")
            nc.vector.scalar_tensor_tensor(
                scrR[:], sqsq[:], 2.0, pe[:], op0=ALU.mult, op1=ALU.mult,
                accum_out=o[:, 2:3])

            # oRe = aE - sum(2 sq^2 pe) on-device -> only 2 output rows
            nc.vector.tensor_sub(o[:, 3:4], o[:, 0:1], o[:, 2:3])
            # block-transpose so the output DMA rows are contiguous:
            # tr[32k+c, p] = o[32k+p, c]
            tr = pc.tile([B, 32], f32, tag="tr")
            nc.vector.transpose(tr[:], o[:])
            # one DMA per quantity row, on different queues: same-queue
            # DMAs of equal shape get mis-merged (dropped transfers)
            nc.gpsimd.dma_start(out_d[0], tr[3:100:32, :])
            nc.sync.dma_start(out_d[1], tr[1:98:32, :])

    nc.compile()

    # Hoist the (wait-free) input DMAs and the FIRST act-table load (Exp
    # set) into the preamble block so they overlap the fixed engine
    # startup.  The mid-kernel Sin-set load stays in the body.
    import os
    mybir_ET = mybir.EngineType
    b0, b1 = nc.main_func.blocks[0], nc.main_func.blocks[1]
    if os.environ.get("NO_HOIST") == "1":
        return nc
    hoist = []
    first_load_seen = False
    for ins in list(b1.instructions):
        nm = type(ins).__name__
        if nm == "InstDMACopy" and ins.engine in (mybir_ET.Pool,
                                                    mybir_ET.SP):
            si = ins.sync_info
            if si is not None and si.on_wait:
                continue  # output DMAs wait on results
            hoist.append(ins)
            b1.instructions.remove(ins)
        elif nm == "InstLoadActFuncSet" and not first_load_seen:
            first_load_seen = True
            si = ins.sync_info
            assert si is None or (not si.on_wait and not si.on_update)
            hoist.append(ins)
            b1.instructions.remove(ins)
    for ins in reversed(hoist):
        first = next((i for i, x in enumerate(b0.instructions)
                      if x.engine == ins.engine), len(b0.instructions))
        b0.instructions.insert(first, ins)
    if os.environ.get("KEEP_DRAIN") != "1":
        for ins in list(b0.instructions):
            if (type(ins).__name__ == "InstDrain"
                    and ins.engine == mybir_ET.Pool):
                b0.instructions.remove(ins)
    # The Sin-set table load sits behind an event-sem wait for the DVE
    # range-reduction ops it does not need; move it (sync-free) directly
    # after the Exp activation so it overlaps the im matmuls + DVE work.
    if os.environ.get("NO_LOAD_EARLY") != "1":
        loads = [i for i, x in enumerate(b1.instructions)
                 if type(x).__name__ == "InstLoadActFuncSet"]
        if loads:
            li = loads[0]
            ld = b1.instructions[li]
            si = ld.sync_info
            assert si is None or (not si.on_wait and not si.on_update)
            exp_i = next(i for i, x in enumerate(b1.instructions)
                         if type(x).__name__ == "InstActivation")
            if li > exp_i + 1:
                b1.instructions.remove(ld)
                b1.instructions.insert(exp_i + 1, ld)
    return nc


def _host_pack(inputs, params_context, inputs_param):
    x = np.asarray(inputs).astype(np.float64)          # (B, L) in {0,1}
    P = np.asarray(params_context)                     # (s, d, m, j) complex
    I = np.asarray(inputs_param)                       # (s, d, m) complex

    mask = (np.arange(L)[None, :] < np.maximum(np.arange(L), 1)[:, None])
    Lp = np.log(P)
    D = (Lp[:, 1] - Lp[:, 0]) * mask[:, None, :]       # (s, m, j)
    C = (Lp[:, 0] * mask[:, None, :]).sum(-1)          # (s, m)
    I0 = I[:, 0]
    I1 = I[:, 1]
    A0 = np.log(np.abs(I0))
    dA = np.log(np.abs(I1)) - A0
    wrap = lambda t: np.angle(np.exp(1j * t))
    ph0 = np.angle(I0)
    dPh = wrap(np.angle(I1) - ph0)
    eye = np.eye(L)[:, None, :]                        # (s, 1, j)
    Dre = D.real + eye * dA[:, :, None]                # (s, m, j)
    Dim = D.imag + eye * dPh[:, :, None]
    CA = C.real + A0 + 0.5 * Dre.sum(-1)               # x-centering shift
    PH = wrap(C.imag + ph0 + 0.5 * Dim.sum(-1))

    xt = np.concatenate([(x - 0.5).T, np.ones((2, B))], 0)  # (66, B)
    rhs_list = []
    for k in range(N_CORES):
        msl = slice(k * NM, (k + 1) * NM)
        full = np.zeros((66, C_TOT), np.float64)
        full[:, C_XT:C_XT + B] = xt
        for Dp, const, chi, clo in ((Dre, CA, C_REH, C_REL),
                                    (Dim, PH, C_IMH, C_IML)):
            Dc = Dp[:, msl, :].transpose(2, 0, 1).reshape(L, NBLK)  # (j, sm)
            Dhi = Dc.astype(_BF16).astype(np.float64)
            full[0:64, chi:chi + NBLK] = Dhi
            full[0:64, clo:clo + NBLK] = Dc - Dhi
            cc = const[:, msl].reshape(NBLK)
            hi = cc.astype(_BF16).astype(np.float64)
            full[64, chi:chi + NBLK] = hi
            full[65, chi:chi + NBLK] = cc - hi
        rhs_list.append(full.astype(_BF16))
    return rhs_list


def kernel(inputs, params_context, inputs_param):
    global _built
    from concourse.bass_utils import run_bass_kernel_spmd

    if _built is None:
        _built = _build()
    nc = _built

    rhs_list = _host_pack(inputs, params_context, inputs_param)
    in_maps = [{"rhs": rhs_list[k]} for k in range(N_CORES)]
    res = run_bass_kernel_spmd(nc, in_maps, list(range(N_CORES)))

    re = np.zeros(B, np.float64)
    im = np.zeros(B, np.float64)
    for k in range(N_CORES):
        q = np.asarray(res.results[k]["out"], np.float64)  # (2, 4, 32)
        re += q[0].reshape(B)
        im += -q[1].reshape(B)
    return (re + 1j * np.angle(np.exp(1j * im))).astype(np.complex128)


# revision 9
# speedup vs baseline: 1.2344x; 1.0936x over previous
"""Trainium2 Bass kernel for the ARqGPSFull autoregressive wavefunction.

Math: out[b] = sum_{s,m} ctx[b,s,m] * I_{x[b,s]}[s,m]; in logs the masked
product is affine in x, so with centered x' = x-0.5 and folded observed-state
selection (see _host_pack):
  T[b,s,m] = exp(Sre) * (cos(Sim) + i sin(Sim)),  S = x'^T D + C  (one matmul)
Per core: 8 of the 64 m-values -> 512 (s,m) columns.  D is shipped as a
bf16 hi+lo pair (two accumulating matmuls, 66+64 contraction rows) so the
matmul is fp32-exact; constants ride rows 64/65 of the hi block split hi/lo.

Engines: exp comes from the ACT Exp table (1e-5 rel); sin from the ACT Sin
table (2e-7) after one mid-kernel table-set switch.  cos(Sim) = 1-2*sin^2
(Sim/2) needs no range reduction (|Sim|<2pi); sin(Sim) = -sin(ths) with
ths = Sim + pi - 2pi*[Sim>0] in [-pi,pi].  Products and the (s,m) sums fuse
into two scalar_tensor_tensor ops with accum_out; Sum(exp) falls out of the
Exp op's accumulator, so the real part is aE - Sum(2 sq^2 pe) on the host.
Outputs (3 per-sample scalars) are stream-transposed on DVE so the output
DMA is 12 contiguous 128B rows instead of 128 tiny descriptors.

Post-compile IR surgery (same trick as before): input DMAs (Pool-queue,
25ns issue each) and the first act-table load are hoisted into the preamble
so they overlap the fixed ~6us engine startup; the second act-table load
(Sin set) stays mid-body.
"""

import sys

for _p in ("/opt/trn_rl_repo", "/root/.axon_site/_ro/trn_rl_repo"):
    if _p not in sys.path:
        sys.path.append(_p)

import math
import numpy as np
import ml_dtypes

N_CORES = 8
B = 128        # batch
L = 64         # n_sites
M = 64         # GPS support dim
NM = M // N_CORES   # m-values per core
NBLK = L * NM  # 512 (s,m) columns per core
PI = math.pi

_BF16 = ml_dtypes.bfloat16

_built = None

# rhs column layout: [REH(512) | XT(128) | REL(512) | IMH(512) | IML(512)]
C_REH = 0
C_XT = C_REH + NBLK
C_REL = C_XT + B
C_IMH = C_REL + NBLK
C_IML = C_IMH + NBLK
C_TOT = C_IML + NBLK


def _build():
    import concourse.bacc as bacc
    import concourse.mybir as mybir
    from concourse import tile

    f32 = mybir.dt.float32
    bf16 = mybir.dt.bfloat16
    AF = mybir.ActivationFunctionType
    ALU = mybir.AluOpType

    nc = bacc.Bacc()
    rhs_d = nc.dram_tensor("rhs", [66, C_TOT], bf16, kind="ExternalInput")
    out_d = nc.dram_tensor("out", [2, 4, 32], f32, kind="ExternalOutput")

    with tile.TileContext(nc) as tc:
        with (
            tc.tile_pool(name="pc", bufs=1) as pc,
            tc.tile_pool(name="psum", bufs=1, space="PSUM") as psum,
        ):
            rhs = pc.tile([66, C_TOT], bf16, tag="rhs")
            # four input DMAs; ordered so the exp path (re blocks) lands
            # first.  Issued on the Pool queue (25ns each vs 565 on SP) and
            # hoisted pre-barrier post-compile.
            nc.sync.dma_start(rhs[:, C_REH:C_REL], rhs_d[:, C_REH:C_REL])
            nc.sync.dma_start(rhs[:, C_REL:C_IMH], rhs_d[:, C_REL:C_IMH])
            nc.gpsimd.dma_start(rhs[:, C_IMH:C_TOT], rhs_d[:, C_IMH:C_TOT])
            xt66 = rhs[:, C_XT:C_XT + B]
            xt64 = rhs[0:64, C_XT:C_XT + B]

            o = pc.tile([B, 32], f32, tag="o")
            nc.gpsimd.memset(o[:], 0.0)

            # S = [Sre(512) | Sim(512)] in PSUM; hi (66 rows, with consts)
            # + lo (64 rows) accumulate per 256-col group.  Emission order
            # mirrors execution order: the tile tick assignment follows it,
            # and out-of-order emission inflates cross-engine waits.
            Sr = psum.tile([B, NBLK], f32, tag="Sr")
            Si = psum.tile([B, NBLK], f32, tag="Si")
            HW = NBLK // 2
            Sre = Sr[:]
            Sim = Si[:]
            pe = pc.tile([B, NBLK], f32, tag="pe")
            w2s = pc.tile([B, NBLK], f32, tag="w2s")
            ths = pc.tile([B, NBLK], f32, tag="ths")
            for g in range(2):   # re chunks first: exp starts earliest
                cs, ce = g * HW, (g + 1) * HW
                nc.tensor.matmul(Sr[:, cs:ce], xt66,
                                 rhs[:, C_REH + cs:C_REH + ce],
                                 start=True, stop=False)
                nc.tensor.matmul(Sr[:, cs:ce], xt64,
                                 rhs[0:64, C_REL + cs:C_REL + ce],
                                 start=False, stop=True)
            # pe = exp(Sre), aE = sum(pe) free from the ACT accumulator
            nc.scalar.activation(pe[:], Sre, AF.Exp, accum_out=o[:, 0:1])
            for g in range(2):
                cs, ce = g * HW, (g + 1) * HW
                nc.tensor.matmul(Si[:, cs:ce], xt66,
                                 rhs[:, C_IMH + cs:C_IMH + ce],
                                 start=True, stop=False)
                nc.tensor.matmul(Si[:, cs:ce], xt64,
                                 rhs[0:64, C_IML + cs:C_IML + ce],
                                 start=False, stop=True)
                # range reduction: ths = Sim + pi - 2pi*[Sim>0]
                nc.vector.tensor_scalar(w2s[:, cs:ce], Sim[:, cs:ce],
                                        0.0, -2 * PI,
                                        op0=ALU.is_gt, op1=ALU.mult)
                nc.vector.scalar_tensor_tensor(
                    ths[:, cs:ce], Sim[:, cs:ce], PI, w2s[:, cs:ce],
                    op0=ALU.add, op1=ALU.add)

            # table switch (inserted by bacc before the first Sin)
            sq = pc.tile([B, NBLK], f32, tag="sq")
            nc.scalar.activation(sq[:], Sim, AF.Sin, scale=0.5)
            sn = pc.tile([B, NBLK], f32, tag="sn")
            nc.scalar.activation(sn[:], ths[:], AF.Sin)

            sqsq = pc.tile([B, NBLK], f32, tag="sqsq")
            nc.vector.tensor_mul(sqsq[:], sq[:], sq[:])

            # o1 = sum(pe * sn) = -Tim ; o2 = sum(2 sq^2 pe) = aE - Tre
            scrI = pc.tile([B, NBLK], f32, tag="scrI")
            nc.vector.scalar_tensor_tensor(
                scrI[:], pe[:], 1.0, sn[:], op0=ALU.mult, op1=ALU.mult,
                accum_out=o[:, 1:2])
            scrR = pc.tile([B, NBLK], f32, tag="scrR")
            nc.vector.scalar_tensor_tensor(
                scrR[:], sqsq[:], 2.0, pe[:], op0=ALU.mult, op1=ALU.mult,
                accum_out=o[:, 2:3])

            # oRe = aE - sum(2 sq^2 pe) on-device -> only 2 output rows
            nc.vector.tensor_sub(o[:, 3:4], o[:, 0:1], o[:, 2:3])
            # block-transpose so the output DMA rows are contiguous:
            # tr[32k+c, p] = o[32k+p, c]
            tr = pc.tile([B, 32], f32, tag="tr")
            nc.vector.transpose(tr[:], o[:])
            # one DMA per quantity row, on different queues: same-queue
            # DMAs of equal shape get mis-merged (dropped transfers)
            nc.gpsimd.dma_start(out_d[0], tr[3:100:32, :])
            nc.sync.dma_start(out_d[1], tr[1:98:32, :])

    nc.compile()

    # Hoist the (wait-free) input DMAs and the FIRST act-table load (Exp
    # set) into the preamble block so they overlap the fixed engine
    # startup.  The mid-kernel Sin-set load stays in the body.
    import os
    mybir_ET = mybir.EngineType
    b0, b1 = nc.main_func.blocks[0], nc.main_func.blocks[1]
    if os.environ.get("NO_HOIST") == "1":
        return nc
    hoist = []
    first_load_seen = False
    for ins in list(b1.instructions):
        nm = type(ins).__name__
        if nm == "InstDMACopy" and ins.engine in (mybir_ET.Pool,
                                                    mybir_ET.SP):
            si = ins.sync_info
            if si is not None and si.on_wait:
                continue  # output DMAs wait on results
            hoist.append(ins)
            b1.instructions.remove(ins)
        elif nm == "InstLoadActFuncSet" and not first_load_seen:
            first_load_seen = True
            si = ins.sync_info
            assert si is None or (not si.on_wait and not si.on_update)
            hoist.append(ins)
            b1.instructions.remove(ins)
    for ins in reversed(hoist):
        first = next((i for i, x in enumerate(b0.instructions)
                      if x.engine == ins.engine), len(b0.instructions))
        b0.instructions.insert(first, ins)
    if os.environ.get("KEEP_DRAIN") != "1":
        for ins in list(b0.instructions):
            if (type(ins).__name__ == "InstDrain"
                    and ins.engine == mybir_ET.Pool):
                b0.instructions.remove(ins)
    # The Sin-set table load sits behind an event-sem wait for the DVE
    # range-reduction ops it does not need; move it (sync-free) directly
    # after the Exp activation so it overlaps the im matmuls + DVE work.
    if os.environ.get("NO_LOAD_EARLY") != "1":
        loads = [i for i, x in enumerate(b1.instructions)
                 if type(x).__name__ == "InstLoadActFuncSet"]
        if loads:
            li = loads[0]
            ld = b1.instructions[li]
            si = ld.sync_info
            assert si is None or (not si.on_wait and not si.on_update)
            exp_i = next(i for i, x in enumerate(b1.instructions)
                         if type(x).__name__ == "InstActivation")
            if li > exp_i + 1:
                b1.instructions.remove(ld)
                b1.instructions.insert(exp_i + 1, ld)
    return nc


def _host_pack(inputs, params_context, inputs_param):
    x = np.asarray(inputs).astype(np.float64)          # (B, L) in {0,1}
    P = np.asarray(params_context)                     # (s, d, m, j) complex
    I = np.asarray(inputs_param)                       # (s, d, m) complex

    mask = (np.arange(L)[None, :] < np.maximum(np.arange(L), 1)[:, None])
    Lp = np.log(P)
    D = (Lp[:, 1] - Lp[:, 0]) * mask[:, None, :]       # (s, m, j)
    C = (Lp[:, 0] * mask[:, None, :]).sum(-1)          # (s, m)
    I0 = I[:, 0]
    I1 = I[:, 1]
    A0 = np.log(np.abs(I0))
    dA = np.log(np.abs(I1)) - A0
    wrap = lambda t: np.angle(np.exp(1j * t))
    ph0 = np.angle(I0)
    dPh = wrap(np.angle(I1) - ph0)
    eye = np.eye(L)[:, None, :]                        # (s, 1, j)
    Dre = D.real + eye * dA[:, :, None]                # (s, m, j)
    Dim = D.imag + eye * dPh[:, :, None]
    CA = C.real + A0 + 0.5 * Dre.sum(-1)               # x-centering shift
    PH = wrap(C.imag + ph0 + 0.5 * Dim.sum(-1))

    xt = np.concatenate([(x - 0.5).T, np.ones((2, B))], 0)  # (66, B)
    rhs_list = []
    for k in range(N_CORES):
        msl = slice(k * NM, (k + 1) * NM)
        full = np.zeros((66, C_TOT), np.float64)
        full[:, C_XT:C_XT + B] = xt
        for Dp, const, chi, clo in ((Dre, CA, C_REH, C_REL),
                                    (Dim, PH, C_IMH, C_IML)):
            Dc = Dp[:, msl, :].transpose(2, 0, 1).reshape(L, NBLK)  # (j, sm)
            Dhi = Dc.astype(_BF16).astype(np.float64)
            full[0:64, chi:chi + NBLK] = Dhi
            full[0:64, clo:clo + NBLK] = Dc - Dhi
            cc = const[:, msl].reshape(NBLK)
            hi = cc.astype(_BF16).astype(np.float64)
            full[64, chi:chi + NBLK] = hi
            full[65, chi:chi + NBLK] = cc - hi
        rhs_list.append(full.astype(_BF16))
    return rhs_list


def kernel(inputs, params_context, inputs_param):
    global _built
    from concourse.bass_utils import run_bass_kernel_spmd

    if _built is None:
        _built = _build()
    nc = _built

    rhs_list = _host_pack(inputs, params_context, inputs_param)
    in_maps = [{"rhs": rhs_list[k]} for k in range(N_CORES)]
    res = run_bass_kernel_spmd(nc, in_maps, list(range(N_CORES)))

    re = np.zeros(B, np.float64)
    im = np.zeros(B, np.float64)
    for k in range(N_CORES):
        q = np.asarray(res.results[k]["out"], np.float64)  # (2, 4, 32)
        re += q[0].reshape(B)
        im += -q[1].reshape(B)
    return (re + 1j * np.angle(np.exp(1j * im))).astype(np.complex128)
